# revision 36
# baseline (speedup 1.0000x reference)
"""Multi-head attention (b=4, n=2048, dim=1024, heads=16, hd=64) on 8 TRN2
NeuronCores.

Sharding: core i = (batch b = i//2, head-half hh = i%2). Each core computes
Q/K/V projections for its 8 heads only (column-split QKV — no duplicated
K/V work), full 2048x2048 attention for those heads, and a row-split
out-projection partial; the host sums the two partials per batch and adds
the (bv-folded) output bias.

Device layouts (feature-major, partition dim first):
  xT   [128, 8 dc, 2048 t]   x^T, d-chunked
  qT   [128, 4 fc, 2048 t]   Q^T local features (head pair p = chunk p)
  kT   [128, 4 fc, 2048 t]   K^T
  v    [128, 16 tt, 8 h, 65] V token-major per head, col 64 == 1.0 (sum row)
  S^T  psum [128 k, 2 h, 512 q] per k-tile: even head rows 0:64, odd 64:128
       of the PE array (tile_position row groups -> concurrent matmuls)
  P~   exp(S^T/8) bf16; PV: po[65, 512] += v_aug.T @ P~ (row 64 = sums)
  attn [128, 4 fc, 2048 t]   normalized, head-concat feature-major
  outT [1024 e, 2048 t] f32  partial (host sums core pairs, adds bias)

Schedule: a stream of 16 (pair, q-chunk) units x 16 k-tile slots. Each slot
emits the two row-tiled score matmuls + exp + previous slot's PV, plus
"filler" projection matmuls popped from a deadline-ordered queue so the PE
never idles long enough for the HAM clock gate to re-throttle. Unit 1 defers
its PV matmuls to its tail so the V-projection (its filler) can complete
under the exp stream instead of in a serial preamble.
"""
import sys

sys.path.insert(0, "/opt/trn_rl_repo")

from collections import deque

import numpy as np
import ml_dtypes

import concourse.bass as bass
import concourse.tile as tile
from concourse import bacc, mybir
from concourse.bass_utils import run_bass_kernel_spmd

BF16 = mybir.dt.bfloat16
F32 = mybir.dt.float32
EXP = mybir.ActivationFunctionType.Exp
MULT = mybir.AluOpType.mult

D = 1024          # model dim
DC = 8            # d chunks of 128
NT = 2048         # tokens per core (q and k)
FL = 512          # local features (8 heads)
FC = 4            # local feature chunks of 128
NH = 8            # local heads
NP = 4            # local head pairs
HD = 64           # head dim
QC = 512          # q chunk (psum free)
NQC = 4           # q chunks
NKT = 16          # k tiles of 128
SB = 2            # heads per score psum tile (even/odd)
N_CORES = 8

_CACHE = {}


def _install_ntff_shim():
    """The agent image's ``antenv`` lacks ``axon_hooks``, so concourse's
    trace=True path can't find the NTFF profile hook even though
    ``libaxon_pjrt.so`` supports it. Recreate the glue (same contract as
    trn_boot's ``_ntff_profile_via_ctypes``)."""
    import types
    import ctypes
    import contextlib

    if "antenv.axon_hooks" in sys.modules:
        return
    so_path = "/opt/axon/libaxon_pjrt.so"
    try:
        lib = ctypes.CDLL(so_path)
        if not hasattr(lib, "axon_start_nrt_profile"):
            return
    except OSError:
        return
    lib.axon_start_nrt_profile.argtypes = [ctypes.POINTER(ctypes.c_int64),
                                           ctypes.c_size_t]
    lib.axon_start_nrt_profile.restype = ctypes.c_int64
    lib.axon_stop_nrt_profile.argtypes = [ctypes.c_char_p]
    lib.axon_stop_nrt_profile.restype = ctypes.c_int64

    @contextlib.contextmanager
    def _hook(output_dir, device_ids):
        import jax
        jax.devices()
        if device_ids:
            ids = (ctypes.c_int64 * len(device_ids))(*device_ids)
            rc = lib.axon_start_nrt_profile(ids, len(device_ids))
        else:
            rc = lib.axon_start_nrt_profile(None, 0)
        if rc != 0:
            raise RuntimeError(f"axon_start_nrt_profile rc={rc}")
        try:
            yield
        finally:
            n = lib.axon_stop_nrt_profile(str(output_dir).encode())
            print(f"ntff profile: {n} file(s) written to {output_dir}",
                  file=sys.stderr)

    mod = types.ModuleType("antenv.axon_hooks")
    _h = [_hook]
    mod.set_axon_ntff_profile_hook = lambda h: _h.__setitem__(0, h)
    mod.get_axon_ntff_profile_hook = lambda: _h[0]
    sys.modules["antenv.axon_hooks"] = mod
    import antenv
    antenv.axon_hooks = mod


def build():
    nc = bacc.Bacc("TRN2", target_bir_lowering=False, debug=False,
                   num_devices=N_CORES)

    # All inputs arrive pre-arranged on the host into the exact device
    # layout (partition dim first, >=4KB contiguous per partition) so DMA
    # descriptors are large and transfers run near peak instead of the
    # ~50GB/s that 1KB DRAM rows yield.
    xT_d = nc.dram_tensor("xT", [128, NQC, DC, QC], BF16,
                          kind="ExternalInput")
    wq_d = nc.dram_tensor("wqT", [128, DC, FL], BF16, kind="ExternalInput")
    wk_d = nc.dram_tensor("wkT", [128, DC, FL], BF16, kind="ExternalInput")
    wv_d = nc.dram_tensor("wvT", [128, DC, FL], BF16, kind="ExternalInput")
    ow_d = nc.dram_tensor("owT", [128, FC, D], BF16, kind="ExternalInput")
    bq_d = nc.dram_tensor("bq", [128, FC], F32, kind="ExternalInput")
    bk_d = nc.dram_tensor("bk", [128, FC], F32, kind="ExternalInput")
    out_d = nc.dram_tensor("outT", [D, NT], F32, kind="ExternalOutput")

    with tile.TileContext(nc) as tc:
        with tc.tile_pool(name="persist", bufs=1) as persist:
            kT = persist.tile([128, FC, NT], BF16)
            qT = persist.tile([128, FC, NT], BF16)
            v = persist.tile([128, NKT, NH, HD + 1], BF16)
            attn = persist.tile([128, FC, NT], BF16)
            bq_sb = persist.tile([128, FC], F32)
            bk_sb = persist.tile([128, FC], F32)
            nc.vector.memset(v[:, :, :, HD:HD + 1], 1.0)
            warm = persist.tile([128, 1], F32)
            nc.vector.memset(warm, 0.0)

            # PSUM budget (8 banks): ps_acc 2x[128,512] proj/out accumulators,
            # ps_s 2x[128,2,512] scores, ps_o 2x[65,512] PV accumulators.
            with tc.tile_pool(name="w1", bufs=1) as w1, \
                 tc.tile_pool(name="xpool", bufs=1) as xpool, \
                 tc.tile_pool(name="ppool", bufs=17) as ppool, \
                 tc.tile_pool(name="nrm", bufs=2) as nrm, \
                 tc.tile_pool(name="fout", bufs=3) as fout, \
                 tc.tile_pool(name="drpool", bufs=4, space="DRAM") as drpool, \
                 tc.tile_pool(name="ps_acc", bufs=2, space="PSUM") as ps_acc, \
                 tc.tile_pool(name="ps_s", bufs=2, space="PSUM") as ps_s, \
                 tc.tile_pool(name="ps_o", bufs=2, space="PSUM") as ps_o:
                xT = xpool.tile([128, NQC, DC, QC], BF16)
                wq = w1.tile([128, DC, FL], BF16, tag="wq")
                wk = w1.tile([128, DC, FL], BF16, tag="wk")
                wv = w1.tile([128, DC, FL], BF16, tag="wv")
                ow = w1.tile([128, FC, D], BF16, tag="ow")

                # Three queues in parallel; every transfer moves >=4KB
                # contiguous per partition (host pre-arranged), so each
                # lands in a few us. scalar: biases+wk (and the warm exp
                # early so ACT reaches exp(0) fast); sync: xT by tc chunk
                # (tc0 first -> first K chain ~5us); gpsimd: wq, wv, ow.
                nc.scalar.dma_start(out=bq_sb, in_=bq_d.ap())
                nc.scalar.dma_start(out=bk_sb, in_=bk_d.ap())
                # dummy exp pulls the ACT_TABLE_LOAD off the first real
                # score tile's critical path
                nc.scalar.activation(warm, warm, EXP)
                nc.scalar.dma_start(out=wk[:, 0:4, :], in_=wk_d.ap()[:, 0:4, :])
                nc.scalar.dma_start(out=wk[:, 4:8, :], in_=wk_d.ap()[:, 4:8, :])
                for tc_i in range(NQC):
                    nc.sync.dma_start(out=xT[:, tc_i, :, :],
                                      in_=xT_d.ap()[:, tc_i, :, :])
                nc.gpsimd.dma_start(out=wq[:, 0:4, :], in_=wq_d.ap()[:, 0:4, :])
                nc.gpsimd.dma_start(out=wq[:, 4:8, :], in_=wq_d.ap()[:, 4:8, :])
                nc.gpsimd.dma_start(out=wv, in_=wv_d.ap())
                nc.gpsimd.dma_start(out=ow, in_=ow_d.ap())

                # ---- projection chains (8 matmuls + epilogue each) ----
                def k_chain(fc, tc_i):
                    tsl = slice(tc_i * QC, (tc_i + 1) * QC)
                    ps = ps_acc.tile([128, QC], F32, tag="ps")
                    for dc in range(DC):
                        yield nc.tensor.matmul(
                            ps, lhsT=wk[:, dc, fc * 128:(fc + 1) * 128],
                            rhs=xT[:, tc_i, dc, :],
                            start=(dc == 0), stop=(dc == DC - 1))
                    yield nc.vector.tensor_scalar_add(
                        kT[:, fc, tsl], ps, bk_sb[:, fc:fc + 1])

                def q_chain(fc, tc_i):
                    tsl = slice(tc_i * QC, (tc_i + 1) * QC)
                    ps = ps_acc.tile([128, QC], F32, tag="ps")
                    for dc in range(DC):
                        yield nc.tensor.matmul(
                            ps, lhsT=wq[:, dc, fc * 128:(fc + 1) * 128],
                            rhs=xT[:, tc_i, dc, :],
                            start=(dc == 0), stop=(dc == DC - 1))
                    yield nc.vector.tensor_scalar_add(
                        qT[:, fc, tsl], ps, bq_sb[:, fc:fc + 1])

                def v_chain(tt):
                    ps = ps_acc.tile([128, QC], F32, tag="ps")
                    for dc in range(DC):
                        yield nc.tensor.matmul(
                            ps,
                            lhsT=xT[:, tt // 4, dc,
                                    (tt % 4) * 128:(tt % 4) * 128 + 128],
                            rhs=wv[:, dc, :],
                            start=(dc == 0), stop=(dc == DC - 1))
                    yield nc.vector.tensor_copy(
                        out=v[:, tt, :, 0:HD],
                        in_=ps.rearrange("p (h d) -> p h d", d=HD))

                def out_chain(ec, tc_i):
                    tsl = slice(tc_i * QC, (tc_i + 1) * QC)
                    ps = ps_acc.tile([128, QC], F32, tag="ps")
                    for fc in range(FC):
                        yield nc.tensor.matmul(
                            ps, lhsT=ow[:, fc, ec * 128:(ec + 1) * 128],
                            rhs=attn[:, fc, tsl],
                            start=(fc == 0), stop=(fc == FC - 1))
                    fo = fout.tile([128, QC], F32, tag="fo")
                    # tc2/tc3 chains run in the tail where ACT is idle and
                    # DVE is busy with the final normalizations — evacuate
                    # there on ACT instead
                    if tc_i >= 2:
                        yield nc.scalar.activation(
                            fo, ps, mybir.ActivationFunctionType.Copy)
                    else:
                        yield nc.vector.tensor_copy(out=fo, in_=ps)
                    # gpsimd queue: keeps the big output transfers from
                    # delaying the normalization DMAs on the sync queue
                    yield nc.gpsimd.dma_start(
                        out=out_d.ap()[ec * 128:(ec + 1) * 128, tsl], in_=fo)

                # Deadline-ordered filler queue of (key, generator); attn
                # units pop a couple of steps per k-tile slot to keep the PE
                # dense while ACT owns the critical path.  Correctness rule:
                # everything a unit's own matmuls READ must be fully emitted
                # before the unit emits them (the PE executes in order, so a
                # score matmul parked on a not-yet-emitted chain's epilogue
                # deadlocks the queue) — require() force-drains those.
                filler = deque()
                done_keys = set()

                def push(key, gen):
                    filler.append((key, gen))

                def drain(n):
                    for _ in range(n):
                        if not filler:
                            return
                        key, gen = filler[0]
                        try:
                            next(gen)
                        except StopIteration:
                            done_keys.add(key)
                            filler.popleft()

                def drain_all():
                    while filler:
                        drain(1)

                def require(*keys):
                    while any(k not in done_keys for k in keys):
                        assert filler, f"missing filler chains: {keys}"
                        drain(1)

                def attn_unit(p, qc, first=False, fill=2, extra=()):
                    # Cascaded schedule: EVERY unit defers its 16 PV matmul
                    # pairs + normalization into the NEXT unit's slots (the
                    # `extra` thunks, flushed two per slot between the gated
                    # score matmuls). This keeps ready PE work between every
                    # exp-gated instruction and moves each unit's PSUM
                    # evacuation safely after its last PV in queue order.
                    # K chunks are required in kt-stages (kt//4 == tc) so the
                    # first scores don't wait on the whole 2048-token K.
                    require(("k", p, 0), ("q", p, qc))
                    if not first:
                        # this unit flushes the previous unit's PV thunks,
                        # which read v: every v chain must be emitted first
                        require(*[("v", tt) for tt in range(NKT)])
                    he, ho = 2 * p, 2 * p + 1
                    qsl = slice(qc * QC, (qc + 1) * QC)
                    po_e = ps_o.tile([HD + 1, QC], F32, tag="po")
                    po_o = ps_o.tile([HD + 1, QC], F32, tag="po")

                    def pv(pt, kt):
                        nc.tensor.matmul(
                            po_e, lhsT=v[:, kt, he, :], rhs=pt[:, 0, :],
                            start=(kt == 0), stop=(kt == NKT - 1))
                        nc.tensor.matmul(
                            po_o, lhsT=v[:, kt, ho, :], rhs=pt[:, 1, :],
                            start=(kt == 0), stop=(kt == NKT - 1))

                    extra = deque(extra)
                    backlog = []
                    for kt in range(NKT):
                        if kt % 4 == 0 and kt > 0:
                            require(("k", p, kt // 4))
                        ss = ps_s.tile([128, SB, QC], F32, tag="ss")
                        for j in range(SB):
                            hi = j * 64
                            nc.tensor.matmul(
                                ss[:, j, :],
                                lhsT=kT[hi:hi + HD, p,
                                        kt * 128:(kt + 1) * 128],
                                rhs=qT[hi:hi + HD, p, qsl],
                                start=True, stop=True)
                        pt = ppool.tile([128, SB, QC], BF16, tag="pt",
                                        bufs=19)
                        nc.scalar.activation(pt, ss, EXP, scale=0.125)
                        backlog.append((pt, kt))
                        for _ in range(2):
                            if extra:
                                extra.popleft()()
                        drain(fill)
                    while extra:
                        extra.popleft()()

                    def norm():
                        _norm(p, qc, po_e, po_o)

                    return ([lambda a=a, b=b: pv(a, b)
                             for a, b in backlog] + [norm])

                def _norm(p, qc, po_e, po_o):
                    # normalization: evacuate both PV accumulators, batch the
                    # two 1/sum rows into one reciprocal, DRAM-bounce the
                    # partition broadcast, multiply.
                    qsl = slice(qc * QC, (qc + 1) * QC)
                    ps_e = nrm.tile([HD + 1, QC], F32, tag="ps_sb", bufs=3)
                    nc.vector.tensor_copy(out=ps_e, in_=po_e)
                    ps_o_sb = nrm.tile([HD + 1, QC], F32, tag="ps_sb",
                                       bufs=3)
                    nc.vector.tensor_copy(out=ps_o_sb, in_=po_o)
                    # partition-gather the two sums rows via DMA (DVE ops
                    # cannot shift partition bases), one reciprocal for both
                    sr = nrm.tile([2, QC], F32, tag="sr")
                    nc.sync.dma_start(out=sr[0:1, :], in_=ps_e[HD:HD + 1, :])
                    nc.sync.dma_start(out=sr[1:2, :],
                                      in_=ps_o_sb[HD:HD + 1, :])
                    rc = nrm.tile([2, QC], F32, tag="rc")
                    nc.vector.reciprocal(rc, sr)
                    dr = drpool.tile([2, QC], F32, tag="dr")
                    nc.sync.dma_start(out=dr, in_=rc)
                    bc_e = nrm.tile([64, QC], F32, tag="bc_e")
                    nc.sync.dma_start(
                        out=bc_e,
                        in_=bass.AP(tensor=dr.tensor, offset=dr.offset,
                                    ap=[[0, 64], dr.ap[-1]]))
                    bc_o = nrm.tile([64, QC], F32, tag="bc_o")
                    nc.sync.dma_start(
                        out=bc_o,
                        in_=bass.AP(tensor=dr.tensor,
                                    offset=dr.offset + QC,
                                    ap=[[0, 64], dr.ap[-1]]))
                    nc.vector.tensor_tensor(
                        out=attn[0:HD, p, qsl],
                        in0=ps_e[0:HD, :], in1=bc_e, op=MULT)
                    sh = nrm.tile([64, QC], BF16, tag="sh")
                    nc.vector.tensor_tensor(
                        out=sh, in0=ps_o_sb[0:HD, :], in1=bc_o, op=MULT)
                    nc.sync.dma_start(out=attn[64:128, p, qsl], in_=sh)

                # ---- emission ----
                # preamble: only K(0, tc0) + Q(0, qc0) gate the first scores
                push(("k", 0, 0), k_chain(0, 0))
                push(("q", 0, 0), q_chain(0, 0))
                require(("k", 0, 0), ("q", 0, 0))

                # unit 1 runs with the remaining K chunks + the V projection
                # as its filler (PV deferred to its tail)
                for tc_i in range(1, NQC):
                    push(("k", 0, tc_i), k_chain(0, tc_i))
                for tt in range(NKT):
                    push(("v", tt), v_chain(tt))
                push(("q", 0, 1), q_chain(0, 1))
                for tc_i in range(NQC):
                    push(("k", 1, tc_i), k_chain(1, tc_i))
                push(("q", 1, 0), q_chain(1, 0))
                bl = attn_unit(0, 0, first=True, fill=9)

                push(("q", 1, 1), q_chain(1, 1))
                push(("q", 0, 2), q_chain(0, 2))
                bl = attn_unit(0, 1, extra=bl)
                push(("q", 0, 3), q_chain(0, 3))
                push(("q", 1, 2), q_chain(1, 2))
                bl = attn_unit(1, 0, extra=bl)
                for tc_i in range(NQC):
                    push(("k", 2, tc_i), k_chain(2, tc_i))
                bl = attn_unit(1, 1, extra=bl)
                push(("q", 1, 3), q_chain(1, 3))
                push(("q", 2, 0), q_chain(2, 0))
                bl = attn_unit(0, 2, extra=bl)
                push(("q", 2, 1), q_chain(2, 1))
                bl = attn_unit(0, 3, extra=bl)
                for tc_i in range(NQC):
                    push(("k", 3, tc_i), k_chain(3, tc_i))
                bl = attn_unit(1, 2, extra=bl)
                push(("q", 3, 0), q_chain(3, 0))
                push(("q", 3, 1), q_chain(3, 1))
                bl = attn_unit(1, 3, extra=bl)
                push(("q", 2, 2), q_chain(2, 2))
                push(("q", 2, 3), q_chain(2, 3))
                bl = attn_unit(2, 0, extra=bl)
                push(("q", 3, 2), q_chain(3, 2))
                push(("q", 3, 3), q_chain(3, 3))
                bl = attn_unit(2, 1, extra=bl)
                bl = attn_unit(3, 0, extra=bl)
                bl = attn_unit(3, 1, extra=bl)
                # qc0 attn for all pairs completes inside unit (3,1) (it
                # flushes (3,0)'s PV+norm) -> out-proj tc0 can follow
                for ec in range(DC):
                    push(("o", ec, 0), out_chain(ec, 0))
                bl = attn_unit(2, 2, extra=bl)
                for ec in range(DC):
                    push(("o", ec, 1), out_chain(ec, 1))
                bl = attn_unit(2, 3, extra=bl)
                bl = attn_unit(3, 2, extra=bl, fill=3)
                bl = attn_unit(3, 3, extra=bl, fill=2)
                for t in bl:          # last unit's PV + normalization
                    t()
                # tc2 out-chains only need (3,2)'s norm (flushed inside
                # (3,3)) — they execute during (3,3)'s normalization
                # latency, keeping the HAM clock gate warm so the tc3
                # tail runs at full clock
                for ec in range(DC):
                    push(("o", ec, 2), out_chain(ec, 2))
                drain_all()
                for ec in range(DC):
                    push(("o", ec, 3), out_chain(ec, 3))
                drain_all()

    nc.compile()
    return nc


def _dev_w(w):
    # [1024 in, F out] -> [128 p, 8 dc, F] partition-major contiguous
    return np.ascontiguousarray(
        w.reshape(DC, 128, w.shape[1]).transpose(1, 0, 2))


def _prep_in_maps(x, qkv_w, qkv_b, out_w, out_b):
    bf = ml_dtypes.bfloat16
    # xT: [1024 d, 2048 t] -> [128 p, 4 tc, 8 dc, 512] so each tc slice is
    # one contiguous-per-partition DMA
    xTs = []
    for b in range(4):
        xt = x[b].T.astype(bf)                       # [1024, 2048]
        xt = xt.reshape(DC, 128, NQC, QC).transpose(1, 2, 0, 3)
        xTs.append(np.ascontiguousarray(xt))
    wqT, wkT, wvT, owT, bq, bk = [], [], [], [], [], []
    for hh in range(2):
        fsl = slice(hh * FL, (hh + 1) * FL)
        wqT.append(_dev_w(qkv_w[0:D][fsl].T.astype(bf)))
        wkT.append(_dev_w(qkv_w[D:2 * D][fsl].T.astype(bf)))
        wvT.append(_dev_w(qkv_w[2 * D:3 * D][fsl].T.astype(bf)))
        ow = out_w.T[fsl].astype(bf)                 # [512 f, 1024 e]
        owT.append(np.ascontiguousarray(
            ow.reshape(FC, 128, D).transpose(1, 0, 2)))
        bq.append(np.ascontiguousarray(
            qkv_b[0:D][fsl].reshape(FC, 128).T).astype(np.float32))
        bk.append(np.ascontiguousarray(
            qkv_b[D:2 * D][fsl].reshape(FC, 128).T).astype(np.float32))

    in_maps = []
    for i in range(N_CORES):
        b, hh = i // 2, i % 2
        in_maps.append(dict(xT=xTs[b], wqT=wqT[hh], wkT=wkT[hh],
                            wvT=wvT[hh], owT=owT[hh], bq=bq[hh], bk=bk[hh]))
    return in_maps


def run(x, qkv_w, qkv_b, out_w, out_b, trace=False):
    if trace:
        _install_ntff_shim()
    if "nc" not in _CACHE:
        _CACHE["nc"] = build()
    nc = _CACHE["nc"]
    x = np.asarray(x, np.float32)
    qkv_w = np.asarray(qkv_w, np.float32)
    qkv_b = np.asarray(qkv_b, np.float32)
    out_w = np.asarray(out_w, np.float32)
    out_b = np.asarray(out_b, np.float32)
    in_maps = _prep_in_maps(x, qkv_w, qkv_b, out_w, out_b)
    res = run_bass_kernel_spmd(nc, in_maps, core_ids=list(range(N_CORES)),
                               trace=trace)
    # host: sum the two head-half partials per batch, add bv-folded bias
    ob_eff = (out_b + out_w @ qkv_b[2 * D:3 * D]).astype(np.float32)
    out = np.empty((4, NT, D), np.float32)
    for b in range(4):
        acc = res.results[2 * b]["outT"] + res.results[2 * b + 1]["outT"]
        out[b] = acc.T + ob_eff
    return out, res


def kernel(**inputs):
    out, _ = run(**inputs)
    return out


# revision 37
# speedup vs baseline: 1.0097x; 1.0097x over previous
"""Multi-head attention (b=4, n=2048, dim=1024, heads=16, hd=64) on 8 TRN2
NeuronCores.

Sharding: core i = (batch b = i//2, head-half hh = i%2). Each core computes
Q/K/V projections for its 8 heads only (column-split QKV — no duplicated
K/V work), full 2048x2048 attention for those heads, and a row-split
out-projection partial; the host sums the two partials per batch and adds
the (bv-folded) output bias.

Device layouts (feature-major, partition dim first):
  xT   [128, 8 dc, 2048 t]   x^T, d-chunked
  qT   [128, 4 fc, 2048 t]   Q^T local features (head pair p = chunk p)
  kT   [128, 4 fc, 2048 t]   K^T
  v    [128, 16 tt, 8 h, 65] V token-major per head, col 64 == 1.0 (sum row)
  S^T  psum [128 k, 2 h, 512 q] per k-tile: even head rows 0:64, odd 64:128
       of the PE array (tile_position row groups -> concurrent matmuls)
  P~   exp(S^T/8) bf16; PV: po[65, 512] += v_aug.T @ P~ (row 64 = sums)
  attn [128, 4 fc, 2048 t]   normalized, head-concat feature-major
  outT [1024 e, 2048 t] f32  partial (host sums core pairs, adds bias)

Schedule: a stream of 16 (pair, q-chunk) units x 16 k-tile slots. Each slot
emits the two row-tiled score matmuls + exp + previous slot's PV, plus
"filler" projection matmuls popped from a deadline-ordered queue so the PE
never idles long enough for the HAM clock gate to re-throttle. Unit 1 defers
its PV matmuls to its tail so the V-projection (its filler) can complete
under the exp stream instead of in a serial preamble.
"""
import sys

sys.path.insert(0, "/opt/trn_rl_repo")

from collections import deque

import numpy as np
import ml_dtypes

import concourse.bass as bass
import concourse.tile as tile
from concourse import bacc, mybir
from concourse.bass_utils import run_bass_kernel_spmd

BF16 = mybir.dt.bfloat16
F32 = mybir.dt.float32
EXP = mybir.ActivationFunctionType.Exp
MULT = mybir.AluOpType.mult

D = 1024          # model dim
DC = 8            # d chunks of 128
NT = 2048         # tokens per core (q and k)
FL = 512          # local features (8 heads)
FC = 4            # local feature chunks of 128
NH = 8            # local heads
NP = 4            # local head pairs
HD = 64           # head dim
QC = 512          # q chunk (psum free)
NQC = 4           # q chunks
NKT = 16          # k tiles of 128
SB = 2            # heads per score psum tile (even/odd)
N_CORES = 8

_CACHE = {}


def _install_ntff_shim():
    """The agent image's ``antenv`` lacks ``axon_hooks``, so concourse's
    trace=True path can't find the NTFF profile hook even though
    ``libaxon_pjrt.so`` supports it. Recreate the glue (same contract as
    trn_boot's ``_ntff_profile_via_ctypes``)."""
    import types
    import ctypes
    import contextlib

    if "antenv.axon_hooks" in sys.modules:
        return
    so_path = "/opt/axon/libaxon_pjrt.so"
    try:
        lib = ctypes.CDLL(so_path)
        if not hasattr(lib, "axon_start_nrt_profile"):
            return
    except OSError:
        return
    lib.axon_start_nrt_profile.argtypes = [ctypes.POINTER(ctypes.c_int64),
                                           ctypes.c_size_t]
    lib.axon_start_nrt_profile.restype = ctypes.c_int64
    lib.axon_stop_nrt_profile.argtypes = [ctypes.c_char_p]
    lib.axon_stop_nrt_profile.restype = ctypes.c_int64

    @contextlib.contextmanager
    def _hook(output_dir, device_ids):
        import jax
        jax.devices()
        if device_ids:
            ids = (ctypes.c_int64 * len(device_ids))(*device_ids)
            rc = lib.axon_start_nrt_profile(ids, len(device_ids))
        else:
            rc = lib.axon_start_nrt_profile(None, 0)
        if rc != 0:
            raise RuntimeError(f"axon_start_nrt_profile rc={rc}")
        try:
            yield
        finally:
            n = lib.axon_stop_nrt_profile(str(output_dir).encode())
            print(f"ntff profile: {n} file(s) written to {output_dir}",
                  file=sys.stderr)

    mod = types.ModuleType("antenv.axon_hooks")
    _h = [_hook]
    mod.set_axon_ntff_profile_hook = lambda h: _h.__setitem__(0, h)
    mod.get_axon_ntff_profile_hook = lambda: _h[0]
    sys.modules["antenv.axon_hooks"] = mod
    import antenv
    antenv.axon_hooks = mod


def build():
    nc = bacc.Bacc("TRN2", target_bir_lowering=False, debug=False,
                   num_devices=N_CORES)

    # All inputs arrive pre-arranged on the host into the exact device
    # layout (partition dim first, >=4KB contiguous per partition) so DMA
    # descriptors are large and transfers run near peak instead of the
    # ~50GB/s that 1KB DRAM rows yield.
    xT_d = nc.dram_tensor("xT", [128, NQC, DC, QC], BF16,
                          kind="ExternalInput")
    wq_d = nc.dram_tensor("wqT", [128, DC, FL], BF16, kind="ExternalInput")
    wk_d = nc.dram_tensor("wkT", [128, DC, FL], BF16, kind="ExternalInput")
    wv_d = nc.dram_tensor("wvT", [128, DC, FL], BF16, kind="ExternalInput")
    ow_d = nc.dram_tensor("owT", [128, FC, D], BF16, kind="ExternalInput")
    bq_d = nc.dram_tensor("bq", [128, FC], F32, kind="ExternalInput")
    bk_d = nc.dram_tensor("bk", [128, FC], F32, kind="ExternalInput")
    out_d = nc.dram_tensor("outT", [D, NT], F32, kind="ExternalOutput")

    with tile.TileContext(nc) as tc:
        with tc.tile_pool(name="persist", bufs=1) as persist:
            kT = persist.tile([128, FC, NT], BF16)
            qT = persist.tile([128, FC, NT], BF16)
            v = persist.tile([128, NKT, NH, HD + 1], BF16)
            attn = persist.tile([128, FC, NT], BF16)
            bq_sb = persist.tile([128, FC], F32)
            bk_sb = persist.tile([128, FC], F32)
            nc.vector.memset(v[:, :, :, HD:HD + 1], 1.0)
            warm = persist.tile([128, 1], F32)
            nc.vector.memset(warm, 0.0)

            # PSUM budget (8 banks): ps_acc 2x[128,512] proj/out accumulators,
            # ps_s 2x[128,2,512] scores, ps_o 2x[65,512] PV accumulators.
            with tc.tile_pool(name="w1", bufs=1) as w1, \
                 tc.tile_pool(name="xpool", bufs=1) as xpool, \
                 tc.tile_pool(name="ppool", bufs=17) as ppool, \
                 tc.tile_pool(name="nrm", bufs=2) as nrm, \
                 tc.tile_pool(name="fout", bufs=3) as fout, \
                 tc.tile_pool(name="drpool", bufs=4, space="DRAM") as drpool, \
                 tc.tile_pool(name="ps_acc", bufs=2, space="PSUM") as ps_acc, \
                 tc.tile_pool(name="ps_s", bufs=2, space="PSUM") as ps_s, \
                 tc.tile_pool(name="ps_o", bufs=2, space="PSUM") as ps_o:
                xT = xpool.tile([128, NQC, DC, QC], BF16)
                wq = w1.tile([128, DC, FL], BF16, tag="wq")
                wk = w1.tile([128, DC, FL], BF16, tag="wk")
                wv = w1.tile([128, DC, FL], BF16, tag="wv")
                ow = w1.tile([128, FC, D], BF16, tag="ow")

                # Three queues in parallel; every transfer moves >=4KB
                # contiguous per partition (host pre-arranged), so each
                # lands in a few us. scalar: biases+wk (and the warm exp
                # early so ACT reaches exp(0) fast); sync: xT by tc chunk
                # (tc0 first -> first K chain ~5us); gpsimd: wq, wv, ow.
                nc.scalar.dma_start(out=bq_sb, in_=bq_d.ap())
                nc.scalar.dma_start(out=bk_sb, in_=bk_d.ap())
                # dummy exp pulls the ACT_TABLE_LOAD off the first real
                # score tile's critical path
                nc.scalar.activation(warm, warm, EXP)
                # dc-pair slices (2KB contiguous per partition) early for
                # progressive availability — the K chain's dc0 matmul can
                # start as soon as the first 256KB lands; coarser later.
                for i in range(4):
                    nc.scalar.dma_start(out=wk[:, 2 * i:2 * i + 2, :],
                                        in_=wk_d.ap()[:, 2 * i:2 * i + 2, :])
                for i in range(4):
                    nc.sync.dma_start(out=xT[:, 0, 2 * i:2 * i + 2, :],
                                      in_=xT_d.ap()[:, 0, 2 * i:2 * i + 2, :])
                for i in range(4):
                    nc.gpsimd.dma_start(out=wq[:, 2 * i:2 * i + 2, :],
                                        in_=wq_d.ap()[:, 2 * i:2 * i + 2, :])
                for h in (slice(0, 4), slice(4, 8)):
                    nc.sync.dma_start(out=xT[:, 1, h, :],
                                      in_=xT_d.ap()[:, 1, h, :])
                nc.gpsimd.dma_start(out=wv[:, 0:4, :], in_=wv_d.ap()[:, 0:4, :])
                nc.gpsimd.dma_start(out=wv[:, 4:8, :], in_=wv_d.ap()[:, 4:8, :])
                nc.sync.dma_start(out=xT[:, 2, :, :], in_=xT_d.ap()[:, 2, :, :])
                nc.sync.dma_start(out=xT[:, 3, :, :], in_=xT_d.ap()[:, 3, :, :])
                nc.gpsimd.dma_start(out=ow, in_=ow_d.ap())

                # ---- projection chains (8 matmuls + epilogue each) ----
                def k_chain(fc, tc_i):
                    tsl = slice(tc_i * QC, (tc_i + 1) * QC)
                    ps = ps_acc.tile([128, QC], F32, tag="ps")
                    for dc in range(DC):
                        yield nc.tensor.matmul(
                            ps, lhsT=wk[:, dc, fc * 128:(fc + 1) * 128],
                            rhs=xT[:, tc_i, dc, :],
                            start=(dc == 0), stop=(dc == DC - 1))
                    yield nc.vector.tensor_scalar_add(
                        kT[:, fc, tsl], ps, bk_sb[:, fc:fc + 1])

                def q_chain(fc, tc_i):
                    tsl = slice(tc_i * QC, (tc_i + 1) * QC)
                    ps = ps_acc.tile([128, QC], F32, tag="ps")
                    for dc in range(DC):
                        yield nc.tensor.matmul(
                            ps, lhsT=wq[:, dc, fc * 128:(fc + 1) * 128],
                            rhs=xT[:, tc_i, dc, :],
                            start=(dc == 0), stop=(dc == DC - 1))
                    yield nc.vector.tensor_scalar_add(
                        qT[:, fc, tsl], ps, bq_sb[:, fc:fc + 1])

                def v_chain(tt):
                    ps = ps_acc.tile([128, QC], F32, tag="ps")
                    for dc in range(DC):
                        yield nc.tensor.matmul(
                            ps,
                            lhsT=xT[:, tt // 4, dc,
                                    (tt % 4) * 128:(tt % 4) * 128 + 128],
                            rhs=wv[:, dc, :],
                            start=(dc == 0), stop=(dc == DC - 1))
                    yield nc.vector.tensor_copy(
                        out=v[:, tt, :, 0:HD],
                        in_=ps.rearrange("p (h d) -> p h d", d=HD))

                def out_chain(ec, tc_i):
                    tsl = slice(tc_i * QC, (tc_i + 1) * QC)
                    ps = ps_acc.tile([128, QC], F32, tag="ps")
                    for fc in range(FC):
                        yield nc.tensor.matmul(
                            ps, lhsT=ow[:, fc, ec * 128:(ec + 1) * 128],
                            rhs=attn[:, fc, tsl],
                            start=(fc == 0), stop=(fc == FC - 1))
                    fo = fout.tile([128, QC], F32, tag="fo")
                    # tc2/tc3 chains run in the tail where ACT is idle and
                    # DVE is busy with the final normalizations — evacuate
                    # there on ACT instead
                    if tc_i >= 2:
                        yield nc.scalar.activation(
                            fo, ps, mybir.ActivationFunctionType.Copy)
                    else:
                        yield nc.vector.tensor_copy(out=fo, in_=ps)
                    # gpsimd queue: keeps the big output transfers from
                    # delaying the normalization DMAs on the sync queue
                    yield nc.gpsimd.dma_start(
                        out=out_d.ap()[ec * 128:(ec + 1) * 128, tsl], in_=fo)

                # Deadline-ordered filler queue of (key, generator); attn
                # units pop a couple of steps per k-tile slot to keep the PE
                # dense while ACT owns the critical path.  Correctness rule:
                # everything a unit's own matmuls READ must be fully emitted
                # before the unit emits them (the PE executes in order, so a
                # score matmul parked on a not-yet-emitted chain's epilogue
                # deadlocks the queue) — require() force-drains those.
                filler = deque()
                done_keys = set()

                def push(key, gen):
                    filler.append((key, gen))

                def drain(n):
                    for _ in range(n):
                        if not filler:
                            return
                        key, gen = filler[0]
                        try:
                            next(gen)
                        except StopIteration:
                            done_keys.add(key)
                            filler.popleft()

                def drain_all():
                    while filler:
                        drain(1)

                def require(*keys):
                    while any(k not in done_keys for k in keys):
                        assert filler, f"missing filler chains: {keys}"
                        drain(1)

                def attn_unit(p, qc, first=False, fill=2, extra=()):
                    # Cascaded schedule: EVERY unit defers its 16 PV matmul
                    # pairs + normalization into the NEXT unit's slots (the
                    # `extra` thunks, flushed two per slot between the gated
                    # score matmuls). This keeps ready PE work between every
                    # exp-gated instruction and moves each unit's PSUM
                    # evacuation safely after its last PV in queue order.
                    # K chunks are required in kt-stages (kt//4 == tc) so the
                    # first scores don't wait on the whole 2048-token K.
                    require(("k", p, 0), ("q", p, qc))
                    if not first:
                        # this unit flushes the previous unit's PV thunks,
                        # which read v: every v chain must be emitted first
                        require(*[("v", tt) for tt in range(NKT)])
                    he, ho = 2 * p, 2 * p + 1
                    qsl = slice(qc * QC, (qc + 1) * QC)
                    po_e = ps_o.tile([HD + 1, QC], F32, tag="po")
                    po_o = ps_o.tile([HD + 1, QC], F32, tag="po")

                    def pv(pt, kt):
                        nc.tensor.matmul(
                            po_e, lhsT=v[:, kt, he, :], rhs=pt[:, 0, :],
                            start=(kt == 0), stop=(kt == NKT - 1))
                        nc.tensor.matmul(
                            po_o, lhsT=v[:, kt, ho, :], rhs=pt[:, 1, :],
                            start=(kt == 0), stop=(kt == NKT - 1))

                    extra = deque(extra)
                    backlog = []
                    for kt in range(NKT):
                        if kt % 4 == 0 and kt > 0:
                            require(("k", p, kt // 4))
                        ss = ps_s.tile([128, SB, QC], F32, tag="ss")
                        for j in range(SB):
                            hi = j * 64
                            nc.tensor.matmul(
                                ss[:, j, :],
                                lhsT=kT[hi:hi + HD, p,
                                        kt * 128:(kt + 1) * 128],
                                rhs=qT[hi:hi + HD, p, qsl],
                                start=True, stop=True)
                        pt = ppool.tile([128, SB, QC], BF16, tag="pt",
                                        bufs=19)
                        nc.scalar.activation(pt, ss, EXP, scale=0.125)
                        backlog.append((pt, kt))
                        for _ in range(2):
                            if extra:
                                extra.popleft()()
                        drain(fill)
                    while extra:
                        extra.popleft()()

                    def norm():
                        _norm(p, qc, po_e, po_o)

                    return ([lambda a=a, b=b: pv(a, b)
                             for a, b in backlog] + [norm])

                def _norm(p, qc, po_e, po_o):
                    # normalization: evacuate both PV accumulators, batch the
                    # two 1/sum rows into one reciprocal, DRAM-bounce the
                    # partition broadcast, multiply.
                    qsl = slice(qc * QC, (qc + 1) * QC)
                    ps_e = nrm.tile([HD + 1, QC], F32, tag="ps_sb", bufs=3)
                    nc.vector.tensor_copy(out=ps_e, in_=po_e)
                    ps_o_sb = nrm.tile([HD + 1, QC], F32, tag="ps_sb",
                                       bufs=3)
                    nc.vector.tensor_copy(out=ps_o_sb, in_=po_o)
                    # partition-gather the two sums rows via DMA (DVE ops
                    # cannot shift partition bases), one reciprocal for both
                    sr = nrm.tile([2, QC], F32, tag="sr")
                    nc.sync.dma_start(out=sr[0:1, :], in_=ps_e[HD:HD + 1, :])
                    nc.sync.dma_start(out=sr[1:2, :],
                                      in_=ps_o_sb[HD:HD + 1, :])
                    rc = nrm.tile([2, QC], F32, tag="rc")
                    nc.vector.reciprocal(rc, sr)
                    dr = drpool.tile([2, QC], F32, tag="dr")
                    nc.sync.dma_start(out=dr, in_=rc)
                    bc_e = nrm.tile([64, QC], F32, tag="bc_e")
                    nc.sync.dma_start(
                        out=bc_e,
                        in_=bass.AP(tensor=dr.tensor, offset=dr.offset,
                                    ap=[[0, 64], dr.ap[-1]]))
                    bc_o = nrm.tile([64, QC], F32, tag="bc_o")
                    nc.sync.dma_start(
                        out=bc_o,
                        in_=bass.AP(tensor=dr.tensor,
                                    offset=dr.offset + QC,
                                    ap=[[0, 64], dr.ap[-1]]))
                    nc.vector.tensor_tensor(
                        out=attn[0:HD, p, qsl],
                        in0=ps_e[0:HD, :], in1=bc_e, op=MULT)
                    sh = nrm.tile([64, QC], BF16, tag="sh")
                    nc.vector.tensor_tensor(
                        out=sh, in0=ps_o_sb[0:HD, :], in1=bc_o, op=MULT)
                    nc.sync.dma_start(out=attn[64:128, p, qsl], in_=sh)

                # ---- emission ----
                # preamble: only K(0, tc0) + Q(0, qc0) gate the first scores
                push(("k", 0, 0), k_chain(0, 0))
                push(("q", 0, 0), q_chain(0, 0))
                require(("k", 0, 0), ("q", 0, 0))

                # unit 1 runs with the remaining K chunks + the V projection
                # as its filler (PV deferred to its tail)
                for tc_i in range(1, NQC):
                    push(("k", 0, tc_i), k_chain(0, tc_i))
                for tt in range(NKT):
                    push(("v", tt), v_chain(tt))
                push(("q", 0, 1), q_chain(0, 1))
                for tc_i in range(NQC):
                    push(("k", 1, tc_i), k_chain(1, tc_i))
                push(("q", 1, 0), q_chain(1, 0))
                bl = attn_unit(0, 0, first=True, fill=9)

                push(("q", 1, 1), q_chain(1, 1))
                push(("q", 0, 2), q_chain(0, 2))
                bl = attn_unit(0, 1, extra=bl)
                push(("q", 0, 3), q_chain(0, 3))
                push(("q", 1, 2), q_chain(1, 2))
                bl = attn_unit(1, 0, extra=bl)
                for tc_i in range(NQC):
                    push(("k", 2, tc_i), k_chain(2, tc_i))
                bl = attn_unit(1, 1, extra=bl)
                push(("q", 1, 3), q_chain(1, 3))
                push(("q", 2, 0), q_chain(2, 0))
                bl = attn_unit(0, 2, extra=bl)
                push(("q", 2, 1), q_chain(2, 1))
                bl = attn_unit(0, 3, extra=bl)
                for tc_i in range(NQC):
                    push(("k", 3, tc_i), k_chain(3, tc_i))
                bl = attn_unit(1, 2, extra=bl)
                push(("q", 3, 0), q_chain(3, 0))
                push(("q", 3, 1), q_chain(3, 1))
                bl = attn_unit(1, 3, extra=bl)
                push(("q", 2, 2), q_chain(2, 2))
                push(("q", 2, 3), q_chain(2, 3))
                bl = attn_unit(2, 0, extra=bl)
                push(("q", 3, 2), q_chain(3, 2))
                push(("q", 3, 3), q_chain(3, 3))
                bl = attn_unit(2, 1, extra=bl)
                bl = attn_unit(3, 0, extra=bl)
                bl = attn_unit(3, 1, extra=bl)
                # qc0 attn for all pairs completes inside unit (3,1) (it
                # flushes (3,0)'s PV+norm) -> out-proj tc0 can follow
                for ec in range(DC):
                    push(("o", ec, 0), out_chain(ec, 0))
                bl = attn_unit(2, 2, extra=bl)
                for ec in range(DC):
                    push(("o", ec, 1), out_chain(ec, 1))
                bl = attn_unit(2, 3, extra=bl)
                bl = attn_unit(3, 2, extra=bl, fill=3)
                bl = attn_unit(3, 3, extra=bl, fill=2)
                for t in bl:          # last unit's PV + normalization
                    t()
                # tc2 out-chains only need (3,2)'s norm (flushed inside
                # (3,3)) — they execute during (3,3)'s normalization
                # latency, keeping the HAM clock gate warm so the tc3
                # tail runs at full clock
                for ec in range(DC):
                    push(("o", ec, 2), out_chain(ec, 2))
                drain_all()
                for ec in range(DC):
                    push(("o", ec, 3), out_chain(ec, 3))
                drain_all()

    nc.compile()
    return nc


def _dev_w(w):
    # [1024 in, F out] -> [128 p, 8 dc, F] partition-major contiguous
    return np.ascontiguousarray(
        w.reshape(DC, 128, w.shape[1]).transpose(1, 0, 2))


def _prep_in_maps(x, qkv_w, qkv_b, out_w, out_b):
    bf = ml_dtypes.bfloat16
    # xT: [1024 d, 2048 t] -> [128 p, 4 tc, 8 dc, 512] so each tc slice is
    # one contiguous-per-partition DMA
    xTs = []
    for b in range(4):
        xt = x[b].T.astype(bf)                       # [1024, 2048]
        xt = xt.reshape(DC, 128, NQC, QC).transpose(1, 2, 0, 3)
        xTs.append(np.ascontiguousarray(xt))
    wqT, wkT, wvT, owT, bq, bk = [], [], [], [], [], []
    for hh in range(2):
        fsl = slice(hh * FL, (hh + 1) * FL)
        wqT.append(_dev_w(qkv_w[0:D][fsl].T.astype(bf)))
        wkT.append(_dev_w(qkv_w[D:2 * D][fsl].T.astype(bf)))
        wvT.append(_dev_w(qkv_w[2 * D:3 * D][fsl].T.astype(bf)))
        ow = out_w.T[fsl].astype(bf)                 # [512 f, 1024 e]
        owT.append(np.ascontiguousarray(
            ow.reshape(FC, 128, D).transpose(1, 0, 2)))
        bq.append(np.ascontiguousarray(
            qkv_b[0:D][fsl].reshape(FC, 128).T).astype(np.float32))
        bk.append(np.ascontiguousarray(
            qkv_b[D:2 * D][fsl].reshape(FC, 128).T).astype(np.float32))

    in_maps = []
    for i in range(N_CORES):
        b, hh = i // 2, i % 2
        in_maps.append(dict(xT=xTs[b], wqT=wqT[hh], wkT=wkT[hh],
                            wvT=wvT[hh], owT=owT[hh], bq=bq[hh], bk=bk[hh]))
    return in_maps


def run(x, qkv_w, qkv_b, out_w, out_b, trace=False):
    if trace:
        _install_ntff_shim()
    if "nc" not in _CACHE:
        _CACHE["nc"] = build()
    nc = _CACHE["nc"]
    x = np.asarray(x, np.float32)
    qkv_w = np.asarray(qkv_w, np.float32)
    qkv_b = np.asarray(qkv_b, np.float32)
    out_w = np.asarray(out_w, np.float32)
    out_b = np.asarray(out_b, np.float32)
    in_maps = _prep_in_maps(x, qkv_w, qkv_b, out_w, out_b)
    res = run_bass_kernel_spmd(nc, in_maps, core_ids=list(range(N_CORES)),
                               trace=trace)
    # host: sum the two head-half partials per batch, add bv-folded bias
    ob_eff = (out_b + out_w @ qkv_b[2 * D:3 * D]).astype(np.float32)
    out = np.empty((4, NT, D), np.float32)
    for b in range(4):
        acc = res.results[2 * b]["outT"] + res.results[2 * b + 1]["outT"]
        out[b] = acc.T + ob_eff
    return out, res


def kernel(**inputs):
    out, _ = run(**inputs)
    return out


# revision 39
# speedup vs baseline: 1.0147x; 1.0049x over previous
"""Multi-head attention (b=4, n=2048, dim=1024, heads=16, hd=64) on 8 TRN2
NeuronCores.

Sharding: core i = (batch b = i//2, head-half hh = i%2). Each core computes
Q/K/V projections for its 8 heads only (column-split QKV — no duplicated
K/V work), full 2048x2048 attention for those heads, and a row-split
out-projection partial; the host sums the two partials per batch and adds
the (bv-folded) output bias.

Device layouts (feature-major, partition dim first):
  xT   [128, 8 dc, 2048 t]   x^T, d-chunked
  qT   [128, 4 fc, 2048 t]   Q^T local features (head pair p = chunk p)
  kT   [128, 4 fc, 2048 t]   K^T
  v    [128, 16 tt, 8 h, 65] V token-major per head, col 64 == 1.0 (sum row)
  S^T  psum [128 k, 2 h, 512 q] per k-tile: even head rows 0:64, odd 64:128
       of the PE array (tile_position row groups -> concurrent matmuls)
  P~   exp(S^T/8) bf16; PV: po[65, 512] += v_aug.T @ P~ (row 64 = sums)
  attn [128, 4 fc, 2048 t]   normalized, head-concat feature-major
  outT [1024 e, 2048 t] f32  partial (host sums core pairs, adds bias)

Schedule: a stream of 16 (pair, q-chunk) units x 16 k-tile slots. Each slot
emits the two row-tiled score matmuls + exp + previous slot's PV, plus
"filler" projection matmuls popped from a deadline-ordered queue so the PE
never idles long enough for the HAM clock gate to re-throttle. Unit 1 defers
its PV matmuls to its tail so the V-projection (its filler) can complete
under the exp stream instead of in a serial preamble.
"""
import sys

sys.path.insert(0, "/opt/trn_rl_repo")

from collections import deque

import numpy as np
import ml_dtypes

import concourse.bass as bass
import concourse.tile as tile
from concourse import bacc, mybir
from concourse.bass_utils import run_bass_kernel_spmd

BF16 = mybir.dt.bfloat16
F32 = mybir.dt.float32
EXP = mybir.ActivationFunctionType.Exp
MULT = mybir.AluOpType.mult

D = 1024          # model dim
DC = 8            # d chunks of 128
NT = 2048         # tokens per core (q and k)
FL = 512          # local features (8 heads)
FC = 4            # local feature chunks of 128
NH = 8            # local heads
NP = 4            # local head pairs
HD = 64           # head dim
QC = 512          # q chunk (psum free)
NQC = 4           # q chunks
NKT = 16          # k tiles of 128
SB = 2            # heads per score psum tile (even/odd)
N_CORES = 8

_CACHE = {}


def _install_ntff_shim():
    """The agent image's ``antenv`` lacks ``axon_hooks``, so concourse's
    trace=True path can't find the NTFF profile hook even though
    ``libaxon_pjrt.so`` supports it. Recreate the glue (same contract as
    trn_boot's ``_ntff_profile_via_ctypes``)."""
    import types
    import ctypes
    import contextlib

    if "antenv.axon_hooks" in sys.modules:
        return
    so_path = "/opt/axon/libaxon_pjrt.so"
    try:
        lib = ctypes.CDLL(so_path)
        if not hasattr(lib, "axon_start_nrt_profile"):
            return
    except OSError:
        return
    lib.axon_start_nrt_profile.argtypes = [ctypes.POINTER(ctypes.c_int64),
                                           ctypes.c_size_t]
    lib.axon_start_nrt_profile.restype = ctypes.c_int64
    lib.axon_stop_nrt_profile.argtypes = [ctypes.c_char_p]
    lib.axon_stop_nrt_profile.restype = ctypes.c_int64

    @contextlib.contextmanager
    def _hook(output_dir, device_ids):
        import jax
        jax.devices()
        if device_ids:
            ids = (ctypes.c_int64 * len(device_ids))(*device_ids)
            rc = lib.axon_start_nrt_profile(ids, len(device_ids))
        else:
            rc = lib.axon_start_nrt_profile(None, 0)
        if rc != 0:
            raise RuntimeError(f"axon_start_nrt_profile rc={rc}")
        try:
            yield
        finally:
            n = lib.axon_stop_nrt_profile(str(output_dir).encode())
            print(f"ntff profile: {n} file(s) written to {output_dir}",
                  file=sys.stderr)

    mod = types.ModuleType("antenv.axon_hooks")
    _h = [_hook]
    mod.set_axon_ntff_profile_hook = lambda h: _h.__setitem__(0, h)
    mod.get_axon_ntff_profile_hook = lambda: _h[0]
    sys.modules["antenv.axon_hooks"] = mod
    import antenv
    antenv.axon_hooks = mod


def build():
    nc = bacc.Bacc("TRN2", target_bir_lowering=False, debug=False,
                   num_devices=N_CORES)

    # All inputs arrive pre-arranged on the host into the exact device
    # layout (partition dim first, >=4KB contiguous per partition) so DMA
    # descriptors are large and transfers run near peak instead of the
    # ~50GB/s that 1KB DRAM rows yield.
    xT_d = nc.dram_tensor("xT", [128, NQC, DC, QC], BF16,
                          kind="ExternalInput")
    wq_d = nc.dram_tensor("wqT", [128, DC, FL], BF16, kind="ExternalInput")
    wk_d = nc.dram_tensor("wkT", [128, DC, FL], BF16, kind="ExternalInput")
    wv_d = nc.dram_tensor("wvT", [128, DC, FL], BF16, kind="ExternalInput")
    ow_d = nc.dram_tensor("owT", [128, FC, D], BF16, kind="ExternalInput")
    bq_d = nc.dram_tensor("bq", [128, FC], F32, kind="ExternalInput")
    bk_d = nc.dram_tensor("bk", [128, FC], F32, kind="ExternalInput")
    out_d = nc.dram_tensor("outT", [D, NT], F32, kind="ExternalOutput")

    with tile.TileContext(nc) as tc:
        with tc.tile_pool(name="persist", bufs=1) as persist:
            kT = persist.tile([128, FC, NT], BF16)
            qT = persist.tile([128, FC, NT], BF16)
            v = persist.tile([128, NKT, NH, HD + 1], BF16)
            attn = persist.tile([128, FC, NT], BF16)
            bq_sb = persist.tile([128, FC], F32)
            bk_sb = persist.tile([128, FC], F32)
            nc.vector.memset(v[:, :, :, HD:HD + 1], 1.0)
            warm = persist.tile([128, 1], F32)
            nc.vector.memset(warm, 0.0)

            # PSUM budget (8 banks): ps_acc 2x[128,512] proj/out accumulators,
            # ps_s 2x[128,2,512] scores, ps_o 2x[65,512] PV accumulators.
            with tc.tile_pool(name="w1", bufs=1) as w1, \
                 tc.tile_pool(name="xpool", bufs=1) as xpool, \
                 tc.tile_pool(name="ppool", bufs=17) as ppool, \
                 tc.tile_pool(name="nrm", bufs=2) as nrm, \
                 tc.tile_pool(name="fout", bufs=3) as fout, \
                 tc.tile_pool(name="drpool", bufs=4, space="DRAM") as drpool, \
                 tc.tile_pool(name="ps_acc", bufs=2, space="PSUM") as ps_acc, \
                 tc.tile_pool(name="ps_s", bufs=2, space="PSUM") as ps_s, \
                 tc.tile_pool(name="ps_o", bufs=2, space="PSUM") as ps_o:
                xT = xpool.tile([128, NQC, DC, QC], BF16)
                wq = w1.tile([128, DC, FL], BF16, tag="wq")
                wk = w1.tile([128, DC, FL], BF16, tag="wk")
                wv = w1.tile([128, DC, FL], BF16, tag="wv")
                ow = w1.tile([128, FC, D], BF16, tag="ow")

                # Three queues in parallel; every transfer moves >=4KB
                # contiguous per partition (host pre-arranged), so each
                # lands in a few us. scalar: biases+wk (and the warm exp
                # early so ACT reaches exp(0) fast); sync: xT by tc chunk
                # (tc0 first -> first K chain ~5us); gpsimd: wq, wv, ow.
                nc.scalar.dma_start(out=bq_sb, in_=bq_d.ap())
                nc.scalar.dma_start(out=bk_sb, in_=bk_d.ap())
                # dummy exp pulls the ACT_TABLE_LOAD off the first real
                # score tile's critical path
                nc.scalar.activation(warm, warm, EXP)
                # dc-pair slices (2KB contiguous per partition) early for
                # progressive availability — the K chain's dc0 matmul can
                # start as soon as the first 256KB lands; coarser later.
                for i in range(4):
                    nc.scalar.dma_start(out=wk[:, 2 * i:2 * i + 2, :],
                                        in_=wk_d.ap()[:, 2 * i:2 * i + 2, :])
                for i in range(4):
                    nc.sync.dma_start(out=xT[:, 0, 2 * i:2 * i + 2, :],
                                      in_=xT_d.ap()[:, 0, 2 * i:2 * i + 2, :])
                for i in range(4):
                    nc.gpsimd.dma_start(out=wq[:, 2 * i:2 * i + 2, :],
                                        in_=wq_d.ap()[:, 2 * i:2 * i + 2, :])
                for h in (slice(0, 4), slice(4, 8)):
                    nc.sync.dma_start(out=xT[:, 1, h, :],
                                      in_=xT_d.ap()[:, 1, h, :])
                nc.gpsimd.dma_start(out=wv[:, 0:4, :], in_=wv_d.ap()[:, 0:4, :])
                nc.gpsimd.dma_start(out=wv[:, 4:8, :], in_=wv_d.ap()[:, 4:8, :])
                nc.sync.dma_start(out=xT[:, 2, :, :], in_=xT_d.ap()[:, 2, :, :])
                nc.sync.dma_start(out=xT[:, 3, :, :], in_=xT_d.ap()[:, 3, :, :])
                nc.gpsimd.dma_start(out=ow, in_=ow_d.ap())

                # ---- projection chains (8 matmuls + epilogue each) ----
                def k_chain(fc, tc_i):
                    tsl = slice(tc_i * QC, (tc_i + 1) * QC)
                    ps = ps_acc.tile([128, QC], F32, tag="ps")
                    for dc in range(DC):
                        yield nc.tensor.matmul(
                            ps, lhsT=wk[:, dc, fc * 128:(fc + 1) * 128],
                            rhs=xT[:, tc_i, dc, :],
                            start=(dc == 0), stop=(dc == DC - 1))
                    yield nc.vector.tensor_scalar_add(
                        kT[:, fc, tsl], ps, bk_sb[:, fc:fc + 1])

                def q_chain(fc, tc_i):
                    tsl = slice(tc_i * QC, (tc_i + 1) * QC)
                    ps = ps_acc.tile([128, QC], F32, tag="ps")
                    for dc in range(DC):
                        yield nc.tensor.matmul(
                            ps, lhsT=wq[:, dc, fc * 128:(fc + 1) * 128],
                            rhs=xT[:, tc_i, dc, :],
                            start=(dc == 0), stop=(dc == DC - 1))
                    yield nc.vector.tensor_scalar_add(
                        qT[:, fc, tsl], ps, bq_sb[:, fc:fc + 1])

                def v_chain(tt):
                    ps = ps_acc.tile([128, QC], F32, tag="ps")
                    for dc in range(DC):
                        yield nc.tensor.matmul(
                            ps,
                            lhsT=xT[:, tt // 4, dc,
                                    (tt % 4) * 128:(tt % 4) * 128 + 128],
                            rhs=wv[:, dc, :],
                            start=(dc == 0), stop=(dc == DC - 1))
                    yield nc.vector.tensor_copy(
                        out=v[:, tt, :, 0:HD],
                        in_=ps.rearrange("p (h d) -> p h d", d=HD))

                def out_chain(ec, tc_i):
                    tsl = slice(tc_i * QC, (tc_i + 1) * QC)
                    ps = ps_acc.tile([128, QC], F32, tag="ps")
                    for fc in range(FC):
                        yield nc.tensor.matmul(
                            ps, lhsT=ow[:, fc, ec * 128:(ec + 1) * 128],
                            rhs=attn[:, fc, tsl],
                            start=(fc == 0), stop=(fc == FC - 1))
                    fo = fout.tile([128, QC], F32, tag="fo")
                    # tc2/tc3 chains run in the tail where ACT is idle and
                    # DVE is busy with the final normalizations — evacuate
                    # there on ACT instead
                    if tc_i >= 2:
                        yield nc.scalar.activation(
                            fo, ps, mybir.ActivationFunctionType.Copy)
                    else:
                        yield nc.vector.tensor_copy(out=fo, in_=ps)
                    # gpsimd queue keeps the big output transfers from
                    # delaying the normalization DMAs on the sync queue;
                    # the final tc3 batch alternates queues so the drain
                    # after the last matmul halves
                    eng = nc.sync if (tc_i == 3 and ec % 2) else nc.gpsimd
                    yield eng.dma_start(
                        out=out_d.ap()[ec * 128:(ec + 1) * 128, tsl], in_=fo)

                # Deadline-ordered filler queue of (key, generator); attn
                # units pop a couple of steps per k-tile slot to keep the PE
                # dense while ACT owns the critical path.  Correctness rule:
                # everything a unit's own matmuls READ must be fully emitted
                # before the unit emits them (the PE executes in order, so a
                # score matmul parked on a not-yet-emitted chain's epilogue
                # deadlocks the queue) — require() force-drains those.
                filler = deque()
                done_keys = set()

                def push(key, gen):
                    filler.append((key, gen))

                def drain(n):
                    for _ in range(n):
                        if not filler:
                            return
                        key, gen = filler[0]
                        try:
                            next(gen)
                        except StopIteration:
                            done_keys.add(key)
                            filler.popleft()

                def drain_all():
                    while filler:
                        drain(1)

                def require(*keys):
                    while any(k not in done_keys for k in keys):
                        assert filler, f"missing filler chains: {keys}"
                        drain(1)

                def attn_unit(p, qc, first=False, fill=2, extra=()):
                    # Cascaded schedule: EVERY unit defers its 16 PV matmul
                    # pairs + normalization into the NEXT unit's slots (the
                    # `extra` thunks, flushed two per slot between the gated
                    # score matmuls). This keeps ready PE work between every
                    # exp-gated instruction and moves each unit's PSUM
                    # evacuation safely after its last PV in queue order.
                    # K chunks are required in kt-stages (kt//4 == tc) so the
                    # first scores don't wait on the whole 2048-token K.
                    require(("k", p, 0), ("q", p, qc))
                    if not first:
                        # this unit flushes the previous unit's PV thunks,
                        # which read v: every v chain must be emitted first
                        require(*[("v", tt) for tt in range(NKT)])
                    he, ho = 2 * p, 2 * p + 1
                    qsl = slice(qc * QC, (qc + 1) * QC)
                    po_e = ps_o.tile([HD + 1, QC], F32, tag="po")
                    po_o = ps_o.tile([HD + 1, QC], F32, tag="po")

                    def pv(pt, kt):
                        nc.tensor.matmul(
                            po_e, lhsT=v[:, kt, he, :], rhs=pt[:, 0, :],
                            start=(kt == 0), stop=(kt == NKT - 1))
                        nc.tensor.matmul(
                            po_o, lhsT=v[:, kt, ho, :], rhs=pt[:, 1, :],
                            start=(kt == 0), stop=(kt == NKT - 1))

                    extra = deque(extra)
                    backlog = []
                    for kt in range(NKT):
                        if kt % 4 == 0 and kt > 0:
                            require(("k", p, kt // 4))
                        ss = ps_s.tile([128, SB, QC], F32, tag="ss")
                        for j in range(SB):
                            hi = j * 64
                            nc.tensor.matmul(
                                ss[:, j, :],
                                lhsT=kT[hi:hi + HD, p,
                                        kt * 128:(kt + 1) * 128],
                                rhs=qT[hi:hi + HD, p, qsl],
                                start=True, stop=True)
                        pt = ppool.tile([128, SB, QC], BF16, tag="pt",
                                        bufs=19)
                        nc.scalar.activation(pt, ss, EXP, scale=0.125)
                        backlog.append((pt, kt))
                        for _ in range(2):
                            if extra:
                                extra.popleft()()
                        drain(fill)
                    while extra:
                        extra.popleft()()

                    def norm():
                        _norm(p, qc, po_e, po_o)

                    return ([lambda a=a, b=b: pv(a, b)
                             for a, b in backlog] + [norm])

                def _norm(p, qc, po_e, po_o):
                    # normalization: evacuate both PV accumulators, batch the
                    # two 1/sum rows into one reciprocal, DRAM-bounce the
                    # partition broadcast, multiply.
                    qsl = slice(qc * QC, (qc + 1) * QC)
                    ps_e = nrm.tile([HD + 1, QC], F32, tag="ps_sb", bufs=3)
                    nc.vector.tensor_copy(out=ps_e, in_=po_e)
                    ps_o_sb = nrm.tile([HD + 1, QC], F32, tag="ps_sb",
                                       bufs=3)
                    nc.vector.tensor_copy(out=ps_o_sb, in_=po_o)
                    # partition-gather the two sums rows via DMA (DVE ops
                    # cannot shift partition bases), one reciprocal for both
                    sr = nrm.tile([2, QC], F32, tag="sr")
                    nc.sync.dma_start(out=sr[0:1, :], in_=ps_e[HD:HD + 1, :])
                    nc.sync.dma_start(out=sr[1:2, :],
                                      in_=ps_o_sb[HD:HD + 1, :])
                    rc = nrm.tile([2, QC], F32, tag="rc")
                    nc.vector.reciprocal(rc, sr)
                    dr = drpool.tile([2, QC], F32, tag="dr")
                    nc.sync.dma_start(out=dr, in_=rc)
                    bc_e = nrm.tile([64, QC], F32, tag="bc_e")
                    nc.sync.dma_start(
                        out=bc_e,
                        in_=bass.AP(tensor=dr.tensor, offset=dr.offset,
                                    ap=[[0, 64], dr.ap[-1]]))
                    bc_o = nrm.tile([64, QC], F32, tag="bc_o")
                    nc.sync.dma_start(
                        out=bc_o,
                        in_=bass.AP(tensor=dr.tensor,
                                    offset=dr.offset + QC,
                                    ap=[[0, 64], dr.ap[-1]]))
                    nc.vector.tensor_tensor(
                        out=attn[0:HD, p, qsl],
                        in0=ps_e[0:HD, :], in1=bc_e, op=MULT)
                    sh = nrm.tile([64, QC], BF16, tag="sh")
                    nc.vector.tensor_tensor(
                        out=sh, in0=ps_o_sb[0:HD, :], in1=bc_o, op=MULT)
                    nc.sync.dma_start(out=attn[64:128, p, qsl], in_=sh)

                # ---- emission ----
                # preamble: only K(0, tc0) + Q(0, qc0) gate the first scores
                push(("k", 0, 0), k_chain(0, 0))
                push(("q", 0, 0), q_chain(0, 0))
                require(("k", 0, 0), ("q", 0, 0))

                # unit 1 runs with the remaining K chunks + the V projection
                # as its filler (PV deferred to its tail)
                for tc_i in range(1, NQC):
                    push(("k", 0, tc_i), k_chain(0, tc_i))
                for tt in range(NKT):
                    push(("v", tt), v_chain(tt))
                push(("q", 0, 1), q_chain(0, 1))
                for tc_i in range(NQC):
                    push(("k", 1, tc_i), k_chain(1, tc_i))
                push(("q", 1, 0), q_chain(1, 0))
                bl = attn_unit(0, 0, first=True, fill=9)

                push(("q", 1, 1), q_chain(1, 1))
                push(("q", 0, 2), q_chain(0, 2))
                bl = attn_unit(0, 1, extra=bl)
                push(("q", 0, 3), q_chain(0, 3))
                push(("q", 1, 2), q_chain(1, 2))
                bl = attn_unit(1, 0, extra=bl)
                for tc_i in range(NQC):
                    push(("k", 2, tc_i), k_chain(2, tc_i))
                bl = attn_unit(1, 1, extra=bl)
                push(("q", 1, 3), q_chain(1, 3))
                push(("q", 2, 0), q_chain(2, 0))
                bl = attn_unit(0, 2, extra=bl)
                push(("q", 2, 1), q_chain(2, 1))
                bl = attn_unit(0, 3, extra=bl)
                for tc_i in range(NQC):
                    push(("k", 3, tc_i), k_chain(3, tc_i))
                bl = attn_unit(1, 2, extra=bl)
                push(("q", 3, 0), q_chain(3, 0))
                push(("q", 3, 1), q_chain(3, 1))
                bl = attn_unit(1, 3, extra=bl)
                push(("q", 2, 2), q_chain(2, 2))
                push(("q", 2, 3), q_chain(2, 3))
                bl = attn_unit(2, 0, extra=bl)
                push(("q", 3, 2), q_chain(3, 2))
                push(("q", 3, 3), q_chain(3, 3))
                bl = attn_unit(2, 1, extra=bl)
                bl = attn_unit(3, 0, extra=bl)
                bl = attn_unit(3, 1, extra=bl)
                # qc0 attn for all pairs completes inside unit (3,1) (it
                # flushes (3,0)'s PV+norm) -> out-proj tc0 can follow
                for ec in range(DC):
                    push(("o", ec, 0), out_chain(ec, 0))
                bl = attn_unit(2, 2, extra=bl)
                for ec in range(DC):
                    push(("o", ec, 1), out_chain(ec, 1))
                bl = attn_unit(2, 3, extra=bl)
                bl = attn_unit(3, 2, extra=bl, fill=3)
                bl = attn_unit(3, 3, extra=bl, fill=2)
                # tc2 out-chains are ready ((3,2)'s norm flushed inside
                # (3,3)) — interleave them with the final PV flush so the
                # PE never idles long enough for the HAM clock gate to
                # re-throttle, and the tc3 tail runs at full clock
                for ec in range(DC):
                    push(("o", ec, 2), out_chain(ec, 2))
                for t in bl:          # last unit's PV + normalization
                    t()
                    drain(3)
                drain_all()
                for ec in range(DC):
                    push(("o", ec, 3), out_chain(ec, 3))
                drain_all()

    nc.compile()
    return nc


def _dev_w(w):
    # [1024 in, F out] -> [128 p, 8 dc, F] partition-major contiguous
    return np.ascontiguousarray(
        w.reshape(DC, 128, w.shape[1]).transpose(1, 0, 2))


def _prep_in_maps(x, qkv_w, qkv_b, out_w, out_b):
    bf = ml_dtypes.bfloat16
    # xT: [1024 d, 2048 t] -> [128 p, 4 tc, 8 dc, 512] so each tc slice is
    # one contiguous-per-partition DMA
    xTs = []
    for b in range(4):
        xt = x[b].T.astype(bf)                       # [1024, 2048]
        xt = xt.reshape(DC, 128, NQC, QC).transpose(1, 2, 0, 3)
        xTs.append(np.ascontiguousarray(xt))
    wqT, wkT, wvT, owT, bq, bk = [], [], [], [], [], []
    for hh in range(2):
        fsl = slice(hh * FL, (hh + 1) * FL)
        wqT.append(_dev_w(qkv_w[0:D][fsl].T.astype(bf)))
        wkT.append(_dev_w(qkv_w[D:2 * D][fsl].T.astype(bf)))
        wvT.append(_dev_w(qkv_w[2 * D:3 * D][fsl].T.astype(bf)))
        ow = out_w.T[fsl].astype(bf)                 # [512 f, 1024 e]
        owT.append(np.ascontiguousarray(
            ow.reshape(FC, 128, D).transpose(1, 0, 2)))
        bq.append(np.ascontiguousarray(
            qkv_b[0:D][fsl].reshape(FC, 128).T).astype(np.float32))
        bk.append(np.ascontiguousarray(
            qkv_b[D:2 * D][fsl].reshape(FC, 128).T).astype(np.float32))

    in_maps = []
    for i in range(N_CORES):
        b, hh = i // 2, i % 2
        in_maps.append(dict(xT=xTs[b], wqT=wqT[hh], wkT=wkT[hh],
                            wvT=wvT[hh], owT=owT[hh], bq=bq[hh], bk=bk[hh]))
    return in_maps


def run(x, qkv_w, qkv_b, out_w, out_b, trace=False):
    if trace:
        _install_ntff_shim()
    if "nc" not in _CACHE:
        _CACHE["nc"] = build()
    nc = _CACHE["nc"]
    x = np.asarray(x, np.float32)
    qkv_w = np.asarray(qkv_w, np.float32)
    qkv_b = np.asarray(qkv_b, np.float32)
    out_w = np.asarray(out_w, np.float32)
    out_b = np.asarray(out_b, np.float32)
    in_maps = _prep_in_maps(x, qkv_w, qkv_b, out_w, out_b)
    res = run_bass_kernel_spmd(nc, in_maps, core_ids=list(range(N_CORES)),
                               trace=trace)
    # host: sum the two head-half partials per batch, add bv-folded bias
    ob_eff = (out_b + out_w @ qkv_b[2 * D:3 * D]).astype(np.float32)
    out = np.empty((4, NT, D), np.float32)
    for b in range(4):
        acc = res.results[2 * b]["outT"] + res.results[2 * b + 1]["outT"]
        out[b] = acc.T + ob_eff
    return out, res


def kernel(**inputs):
    out, _ = run(**inputs)
    return out
